# revision 22
# baseline (speedup 1.0000x reference)
"""Trainium2 Bass kernel for NanochatAttention (sliding-window GQA attention).

Sharding: 8 cores = (batch b in {0,1}) x (kv-group g in {0..3}).
Each core handles one batch's full sequence for one KV head and its 4 Q heads:
projections, RoPE + QK RMS-norm, value-embedding gate, 512-window causal
attention, and the row-parallel out-projection slice -> partial [T, E] output.
Host sums the 4 partials per batch at unshard time.

v5 highlights (evidence-driven, see trace history):
  * RMS rsqrt = ACT Sqrt + DVE reciprocal.  Square/Sqrt/Copy share ONE
    activation-table set, Exp (phase B) is the only other set -> 2 table
    loads total.  (v2's Ln+Exp rsqrt alternated two sets: 33 loads,
    ~2.6us/tile of Scalar-engine stall that backed up the PE.)
  * DMA: few big issues (each sync dma_start costs ~0.6us of sync-engine
    time).  Weights + the first 256 t-cols of x go in 4-slab groups so
    tile-0's projection chain starts as soon as group 0 lands; x remainder
    streams in 256-col slices ahead of the per-tile compute.
  * Out-projection lags one tile and stages per-512-col chunks (3 DVE /
    1 ACT copies).  NOTE measured dead ends: DMA straight from PSUM is
    not allowed (dma_start src must be SBUF); interleaving the staging
    copies between y-chain heads + grouped 4-slab weight DMAs DID fill
    the PE gaps (43us idle vs 59us) but made every matmul ~20% slower
    from SBUF port contention - net 274us vs 256us.  Less concurrency
    won here.
  * fp8 DoubleRow was tried (v4) and reverted: with --enable-ldw-opt=false
    every matmul pays its own LDWEIGHTS, so 3x more 256-col instructions
    lose to bf16's 512-col streams (314us vs 256us).
  * tensor_tensor_reduce / Pool-engine ops crash this device - avoided.

Attention computes TRANSPOSED scores ST[j, i] = k_j . q_i directly
(stationary = kT block, moving = all 4 heads' scaled qT), so the exp
output IS P^T in SBUF (no PE transpose matmuls for P).  Row sums come
from a ones-column appended to V (PV matmul streams 129 cols; col 128
accumulates sum_j P[i, j]).  Softmax normalization is applied
per-partition while copying the PV accumulator out of PSUM; an identity-
transpose per (tile, head) yields yT for the out-projection.
"""

import numpy as np
import ml_dtypes

import concourse.bass as bass
import concourse.bacc as bacc
import concourse.tile as tile
from concourse import mybir
from concourse import bass_utils

BF = mybir.dt.bfloat16
F32 = mybir.dt.float32
AF = mybir.ActivationFunctionType
ALU = mybir.AluOpType

B = 2
T = 2048
E = 2048
D = 128          # head dim
HQ = 4           # q heads per core (one kv group)
NKV = 4
NT = T // 128    # 16 t-tiles
NE = E // 128    # 16 e-tiles
W = 512          # sliding window
NJB = W // 128   # history blocks
EPS = float(np.finfo(np.float32).eps)


def _bcast_mid(ap, n):
    """Insert a step-0 dim after the partition dim: [p, w] -> [p, n, w]."""
    return bass.AP(tensor=ap.tensor, offset=ap.offset,
                   ap=[ap.ap[0], [0, n], *ap.ap[1:]])


def _half_swap(ap2d, nmid):
    """[p, nmid*128] -> [p, nmid, 2, 64] view with the 64-halves swapped."""
    return bass.AP(tensor=ap2d.tensor, offset=ap2d.offset + 64,
                   ap=[ap2d.ap[0], [128, nmid], [-64, 2], [1, 64]])


def _body(tc, io):
    nc = tc.nc
    xT, wq, wkvg, wo, ve2, cosd, sind, masks, ident, out = (
        io["xT"], io["wq"], io["wkvg"], io["wo"], io["ve2"], io["cos"],
        io["sin"], io["masks"], io["ident"], io["out"])

    with (
        tc.tile_pool(name="const", bufs=1) as cpool,
        tc.tile_pool(name="state", bufs=1) as state,
    ):
        ident_sb = cpool.tile([128, 128], BF, tag="ident")
        nc.sync.dma_start(ident_sb, ident)
        cos_sb = cpool.tile([128, NT, 128], BF, tag="cos")
        sin_sb = cpool.tile([128, NT, 128], BF, tag="sin")
        ve_sb = cpool.tile([128, NT, D], BF, tag="ve")
        cosr = cosd.rearrange("(t p) h -> p t h", p=128)
        sinr = sind.rearrange("(t p) h -> p t h", p=128)
        ver = ve2.rearrange("(t p) d -> p t d", p=128)
        nc.sync.dma_start(cos_sb[:, 0:2, :], cosr[:, 0:2, :])
        nc.sync.dma_start(sin_sb[:, 0:2, :], sinr[:, 0:2, :])
        nc.sync.dma_start(ve_sb[:, 0:2, :], ver[:, 0:2, :])

        wqkvg_sb = cpool.tile([128, NE, 768], BF, tag="wqkvg")
        wqd = wq.rearrange("(e p) f -> p e f", p=128)
        wkvgd = wkvg.rearrange("(e p) f -> p e f", p=128)
        xTr = xT.rearrange("(e p) t -> p e t", p=128)
        with tc.tile_pool(name="xp", bufs=1) as xp:
            xT_sb = xp.tile([128, NE, T], BF, tag="xT")
            # weights + the first 256 t-cols of x, per contraction slab:
            # tile-0/1 projections start as soon as slab 0 lands.
            for e in range(NE):
                nc.sync.dma_start(wqkvg_sb[:, e, 0:512], wqd[:, e])
                nc.sync.dma_start(wqkvg_sb[:, e, 512:768], wkvgd[:, e])
                nc.sync.dma_start(xT_sb[:, e, 0:256], xTr[:, e, 0:256])
            # x slices for tiles 2-5 BEFORE the small-tensor remainder:
            # the measured 6.8us PE stall at t~44us was phase A waiting
            # for slice tb=1 behind the cos/sin/ve bytes.
            for tb in (1, 2):
                cs = slice(tb * 256, (tb + 1) * 256)
                nc.sync.dma_start(xT_sb[:, :, cs], xTr[:, :, cs])
            nc.sync.dma_start(cos_sb[:, 2:, :], cosr[:, 2:, :])
            nc.sync.dma_start(sin_sb[:, 2:, :], sinr[:, 2:, :])
            nc.sync.dma_start(ve_sb[:, 2:, :], ver[:, 2:, :])
            for tb in range(3, T // 256):
                cs = slice(tb * 256, (tb + 1) * 256)
                nc.sync.dma_start(xT_sb[:, :, cs], xTr[:, :, cs])
            masks_sb = cpool.tile([128, 2, 128], BF, tag="masks")
            nc.sync.dma_start(masks_sb, masks.rearrange("m p j -> p m j"))
            wo_sb = cpool.tile([128, HQ, E], BF, tag="wo")
            nc.sync.dma_start(wo_sb, wo.rearrange("(h p) e -> p h e", p=128))

            # ---- whole-sequence state ------------------------------------
            qkT_all = state.tile([128, NT, HQ, 128], BF, tag="qkT")
            kT_all = state.tile([128, NT, 128], BF, tag="kT")
            v_all = state.tile([128, NT, 129], BF, tag="v")
            yT_all = state.tile([128, NT, HQ, 128], BF, tag="yT")

            # ones column for PV row sums (independent of v_all's data cols)
            nc.vector.tensor_scalar(
                v_all[:, :, 128:129].rearrange("p a b -> p (a b)"),
                ident_sb[:, 0:NT], 0.0, 1.0, op0=ALU.mult, op1=ALU.add)

            # ============ PHASE A: proj + rope + rms + transposes =========
            with (
                tc.tile_pool(name="workA", bufs=3) as work,
                tc.tile_pool(name="psA_q", bufs=4, space="PSUM") as psA_q,
                tc.tile_pool(name="psA_kvg", bufs=2, space="PSUM") as psA_kvg,
                tc.tile_pool(name="psA_tr", bufs=1, space="PSUM") as psA_tr,
            ):
                def _emit_transposes(ptt, qros_p, krms_p):
                    trp = psA_tr.tile([128, HQ + 1, 128], F32, tag="trp")
                    for h in range(HQ):
                        nc.tensor.matmul(trp[:, h, :], qros_p[:, h, :],
                                         ident_sb, start=True, stop=True)
                    nc.tensor.matmul(trp[:, HQ, :], krms_p, ident_sb,
                                     start=True, stop=True)
                    # qT copy on ACT (Copy is a filler in every table set)
                    nc.scalar.activation(
                        qkT_all[:, ptt, :, :].rearrange("p a b -> p (a b)"),
                        trp[:, 0:HQ, :].rearrange("p a b -> p (a b)"),
                        AF.Copy)
                    nc.vector.tensor_copy(kT_all[:, ptt, :], trp[:, HQ, :])

                lagq = []
                for tt in range(NT):
                    ts = slice(tt * 128, (tt + 1) * 128)
                    psq = psA_q.tile([128, 512], F32, tag="psq")
                    pskvg = psA_kvg.tile([128, 256], F32, tag="pskvg")
                    for e in range(NE):
                        nc.tensor.matmul(psq, xT_sb[:, e, ts],
                                         wqkvg_sb[:, e, 0:512],
                                         start=(e == 0), stop=(e == NE - 1))
                    for e in range(NE):
                        nc.tensor.matmul(pskvg, xT_sb[:, e, ts],
                                         wqkvg_sb[:, e, 512:768],
                                         start=(e == 0), stop=(e == NE - 1))

                    # v = v_proj + gated ve (gate pre-applied on host)
                    nc.vector.tensor_add(v_all[:, tt, 0:128],
                                         pskvg[:, 128:256], ve_sb[:, tt, :])

                    # cast projections to bf16, then RoPE in fast DVE modes
                    qb = work.tile([128, HQ * D], BF, tag="qb")
                    nc.vector.tensor_copy(qb, psq)
                    kb = work.tile([128, D], BF, tag="kb")
                    nc.vector.tensor_copy(kb, pskvg[:, 0:128])

                    cosq = _bcast_mid(cos_sb[:, tt, :], HQ)
                    sinq = _bcast_mid(sin_sb[:, tt, :], HQ)
                    qro = work.tile([128, HQ, D], BF, tag="qro")
                    rb = work.tile([128, HQ, D], BF, tag="rb")
                    nc.vector.tensor_mul(
                        qro, qb.rearrange("p (h d) -> p h d", h=HQ), cosq)
                    nc.vector.tensor_mul(
                        rb.rearrange("p h (s x) -> p h s x", s=2),
                        _half_swap(qb[:], HQ),
                        sinq.rearrange("p h (s x) -> p h s x", s=2))
                    nc.vector.tensor_add(qro, qro, rb)

                    kro = work.tile([128, D], BF, tag="kro")
                    kb2 = work.tile([128, D], BF, tag="kb2")
                    nc.vector.tensor_mul(kro, kb, cos_sb[:, tt, :])
                    nc.vector.tensor_mul(
                        kb2.rearrange("p (s x) -> p s x", s=2),
                        _half_swap(kb[:], 1),
                        sin_sb[:, tt, :].rearrange("p (s x) -> p s x", s=2))
                    nc.vector.tensor_add(kro, kro, kb2)

                    # RMS sums of squares (ACT Square + accum)
                    sq = work.tile([128, (HQ + 1) * D], BF, tag="sq")
                    qkss = work.tile([128, HQ + 1], F32, tag="qkss")
                    for h in range(HQ):
                        nc.scalar.activation(sq[:, h * D:(h + 1) * D],
                                             qro[:, h, :], AF.Square,
                                             accum_out=qkss[:, h:h + 1])
                    nc.scalar.activation(sq[:, HQ * D:], kro, AF.Square,
                                         accum_out=qkss[:, HQ:HQ + 1])
                    # rsqrt = DVE reciprocal(ACT Sqrt); the k column folds
                    # the sqrt(D) score scale via a 1/D pre-multiply.
                    tmp5 = work.tile([128, HQ + 1], F32, tag="tmp5")
                    nc.vector.tensor_scalar_add(tmp5[:, 0:HQ],
                                                qkss[:, 0:HQ],
                                                float(128.0 * EPS))
                    nc.vector.tensor_scalar(tmp5[:, HQ:HQ + 1],
                                            qkss[:, HQ:HQ + 1],
                                            float(1.0 / 128.0), EPS,
                                            op0=ALU.mult, op1=ALU.add)
                    rt5 = work.tile([128, HQ + 1], F32, tag="rt5")
                    nc.scalar.activation(rt5, tmp5, AF.Sqrt)
                    sc5 = work.tile([128, HQ + 1], F32, tag="sc5")
                    nc.vector.reciprocal(sc5, rt5)

                    # krms = kro * sc5_k ; qros_h = qro_h * sc5_h (DVE)
                    krms = work.tile([128, D], BF, tag="krms")
                    nc.vector.tensor_scalar_mul(krms, kro,
                                                sc5[:, HQ:HQ + 1])
                    qros = work.tile([128, HQ, D], BF, tag="qros")
                    for h in range(HQ):
                        nc.vector.tensor_scalar_mul(qros[:, h, :],
                                                    qro[:, h, :],
                                                    sc5[:, h:h + 1])

                    # transposes run TWO tiles lagged so the PE never
                    # waits on the DVE/ACT chain
                    lagq.append((tt, qros, krms))
                    if len(lagq) > 2:
                        _emit_transposes(*lagq.pop(0))
                for item in lagq:
                    _emit_transposes(*item)

        # ============= PHASE B: attention ============================
        with (
            tc.tile_pool(name="attn", bufs=3) as attn,
            tc.tile_pool(name="outc", bufs=3) as outc,
            tc.tile_pool(name="psB", bufs=3, space="PSUM") as psB,
            tc.tile_pool(name="psY", bufs=2, space="PSUM") as psY,
            tc.tile_pool(name="psC", bufs=3, space="PSUM") as psC,
        ):
            def _emit_outproj(ptt):
                # four 512-col chunks, each DMA'd as soon as it lands
                pts = slice(ptt * 128, (ptt + 1) * 128)
                for ec in range(4):
                    ops = psC.tile([128, 512], F32, tag="ops")
                    for h in range(HQ):
                        nc.tensor.matmul(
                            ops, yT_all[:, ptt, h, :],
                            wo_sb[:, h, ec * 512:(ec + 1) * 512],
                            start=(h == 0), stop=(h == HQ - 1))
                    osb = outc.tile([128, 512], BF, tag="osb")
                    if ec == 3:
                        nc.scalar.activation(osb, ops, AF.Copy)
                    else:
                        nc.vector.tensor_copy(osb, ops)
                    nc.sync.dma_start(out[pts, ec * 512:(ec + 1) * 512],
                                      osb)

            for tt in range(NT):
                njb = min(tt, NJB) + 1
                jb0 = tt - (njb - 1)
                nhist = njb - 1
                # scores for ALL 4 heads per key block: one 512-col
                # matmul (stationary kT block, moving all heads' qT),
                # exp'd per block.  Masks are applied POST-exp as
                # multiplicative 0/1 bf16 on the DVE.
                pexp = attn.tile([128, NJB + 3, HQ, 128], BF, tag="pexp")
                slot = list(range(njb))
                for jb in range(njb):
                    st = psB.tile([128, HQ, 128], F32, tag="st")
                    nc.tensor.matmul(
                        st.rearrange("p h i -> p (h i)"),
                        kT_all[:, jb0 + jb, :],
                        qkT_all[:, tt, :, :].rearrange(
                            "p h i -> p (h i)"),
                        start=True, stop=True)
                    nc.scalar.activation(
                        pexp[:, jb, :, :].rearrange("p h i -> p (h i)"),
                        st.rearrange("p h i -> p (h i)"), AF.Exp)
                    if jb == nhist:
                        nc.vector.tensor_mul(
                            pexp[:, NJB + 2, :, :], pexp[:, jb, :, :],
                            _bcast_mid(masks_sb[:, 1, :], HQ))
                        slot[jb] = NJB + 2
                    elif jb == 0 and tt >= NJB:
                        nc.vector.tensor_mul(
                            pexp[:, NJB + 1, :, :], pexp[:, jb, :, :],
                            _bcast_mid(masks_sb[:, 0, :], HQ))
                        slot[jb] = NJB + 1

                for h in range(HQ):
                    # PV with ones column: col 128 = row sums.  The
                    # y tile also hosts the yT transpose result in cols
                    # 129:257 so both fit one PSUM bank.
                    y_ps = psY.tile([128, 257], F32, tag="y")
                    for jb in range(njb):
                        nc.tensor.matmul(y_ps[:, 0:129],
                                         pexp[:, slot[jb], h, :],
                                         v_all[:, jb0 + jb, :],
                                         start=(jb == 0),
                                         stop=(jb == njb - 1))
                    rsum = attn.tile([128, 1], F32, tag="rsum")
                    nc.vector.reciprocal(rsum, y_ps[:, 128:129])
                    y_sb = attn.tile([128, 128], BF, tag="ysb")
                    if h % 2 == 0:
                        nc.vector.tensor_scalar_mul(y_sb,
                                                    y_ps[:, 0:128],
                                                    rsum)
                    else:
                        nc.scalar.activation(y_sb, y_ps[:, 0:128],
                                             AF.Copy, scale=rsum)
                    # transpose y -> yT
                    nc.tensor.matmul(y_ps[:, 129:257], y_sb, ident_sb,
                                     start=True, stop=True)
                    nc.vector.tensor_copy(yT_all[:, tt, h, :],
                                          y_ps[:, 129:257])

                # out-projection runs one tile LAGGED so the PE
                # never stalls on this tile's y normalization chain
                if tt > 0:
                    _emit_outproj(tt - 1)
            _emit_outproj(NT - 1)


def build_nc(stage=99):
    nc = bacc.Bacc("TRN2", target_bir_lowering=False, debug=False,
                   num_devices=8)
    io = {
        "xT": nc.dram_tensor("xT", [E, T], BF, kind="ExternalInput").ap(),
        "wq": nc.dram_tensor("wq", [E, HQ * D], BF, kind="ExternalInput").ap(),
        "wkvg": nc.dram_tensor("wkvg", [E, 256], BF, kind="ExternalInput").ap(),
        "wo": nc.dram_tensor("wo", [HQ * D, E], BF, kind="ExternalInput").ap(),
        "ve2": nc.dram_tensor("ve2", [T, D], BF, kind="ExternalInput").ap(),
        "cos": nc.dram_tensor("cos", [T, 128], BF, kind="ExternalInput").ap(),
        "sin": nc.dram_tensor("sin", [T, 128], BF, kind="ExternalInput").ap(),
        "masks": nc.dram_tensor("masks", [2, 128, 128], BF,
                                kind="ExternalInput").ap(),
        "ident": nc.dram_tensor("ident", [128, 128], BF,
                                kind="ExternalInput").ap(),
        "out": nc.dram_tensor("out", [T, E], BF, kind="ExternalOutput").ap(),
    }
    with tile.TileContext(nc) as tc:
        _body(tc, io)
    nc.compile()
    return nc


_NC = None


def _get_nc():
    global _NC
    if _NC is None:
        _NC = build_nc()
    return _NC


def _prep_in_maps(x, ve, cos, sin, wq, wk, wv, wo, wgate):
    x = np.asarray(x, dtype=np.float32)
    ve = np.asarray(ve, dtype=np.float32)
    cos1 = np.asarray(cos, np.float32).reshape(T, 64)
    sin1 = np.asarray(sin, np.float32).reshape(T, 64)
    cos2 = np.ascontiguousarray(
        np.concatenate([cos1, cos1], axis=1)).astype(ml_dtypes.bfloat16)
    sin2 = np.ascontiguousarray(
        np.concatenate([sin1, -sin1], axis=1)).astype(ml_dtypes.bfloat16)
    ii = np.arange(128)
    masks = np.ones((2, 128, 128), np.float32)
    # multiplicative post-exp masks, transposed for the P^T[j, i] layout:
    # window-edge block: kill keys j <= i ; causal diag block: kill j > i
    masks[0][ii[:, None] <= ii[None, :]] = 0.0
    masks[1][ii[:, None] > ii[None, :]] = 0.0
    masks = np.ascontiguousarray(masks).astype(ml_dtypes.bfloat16)
    ident = np.eye(128, dtype=ml_dtypes.bfloat16)

    xT_b = [np.ascontiguousarray(x[b].T).astype(ml_dtypes.bfloat16)
            for b in range(B)]
    in_maps = []
    for c in range(8):
        b, g = divmod(c, NKV)
        wq_c = np.ascontiguousarray(
            wq[g * 512:(g + 1) * 512, :].T).astype(ml_dtypes.bfloat16)
        wk_c = wk[g * 128:(g + 1) * 128, :].T
        wv_c = wv[g * 128:(g + 1) * 128, :].T
        wkvg_c = np.ascontiguousarray(
            np.concatenate([wk_c, wv_c], axis=1)).astype(ml_dtypes.bfloat16)
        wo_c = np.ascontiguousarray(
            wo[:, g * 512:(g + 1) * 512].T).astype(ml_dtypes.bfloat16)
        # value-embedding gate folded on the host (tiny 32-ch matmul)
        logit = x[b, :, :32] @ wgate[g]
        gate = 2.0 / (1.0 + np.exp(-logit))
        ve2_c = np.ascontiguousarray(
            gate[:, None] * ve[b, :, g * 128:(g + 1) * 128]).astype(
                ml_dtypes.bfloat16)
        in_maps.append({
            "xT": xT_b[b], "wq": wq_c, "wkvg": wkvg_c, "wo": wo_c,
            "ve2": ve2_c, "cos": cos2, "sin": sin2, "masks": masks,
            "ident": ident,
        })
    return in_maps


def kernel(x, ve, cos, sin, wq, wk, wv, wo, wgate, window_size=512,
           _trace=False):
    assert int(window_size) == W, f"kernel hardcodes window {W}"
    wq = np.asarray(wq, np.float32)
    wk = np.asarray(wk, np.float32)
    wv = np.asarray(wv, np.float32)
    wo = np.asarray(wo, np.float32)
    wgate = np.asarray(wgate, np.float32)
    in_maps = _prep_in_maps(x, ve, cos, sin, wq, wk, wv, wo, wgate)
    nc = _get_nc()
    res = bass_utils.run_bass_kernel_spmd(
        nc, in_maps, core_ids=list(range(8)), trace=_trace)
    out = np.empty((B, T, E), np.float32)
    for b in range(B):
        acc = res.results[b * NKV]["out"].astype(np.float32).copy()
        for g in range(1, NKV):
            acc += res.results[b * NKV + g]["out"]
        out[b] = acc
    if _trace:
        kernel.last_results = res
    return out


# revision 25
# speedup vs baseline: 1.1382x; 1.1382x over previous
"""Trainium2 Bass kernel for NanochatAttention (sliding-window GQA attention).

Sharding: 8 cores = (batch b in {0,1}) x (kv-group g in {0..3}).
Each core handles one batch's full sequence for one KV head and its 4 Q heads:
projections, RoPE + QK RMS-norm, value-embedding gate, 512-window causal
attention, and the row-parallel out-projection slice -> partial [T, E] output.
Host sums the 4 partials per batch at unshard time.

v5 highlights (evidence-driven, see trace history):
  * RMS rsqrt = ACT Sqrt + DVE reciprocal.  Square/Sqrt/Copy share ONE
    activation-table set, Exp (phase B) is the only other set -> 2 table
    loads total.  (v2's Ln+Exp rsqrt alternated two sets: 33 loads,
    ~2.6us/tile of Scalar-engine stall that backed up the PE.)
  * DMA: few big issues (each sync dma_start costs ~0.6us of sync-engine
    time).  Weights + the first 256 t-cols of x go in 4-slab groups so
    tile-0's projection chain starts as soon as group 0 lands; x remainder
    streams in 256-col slices ahead of the per-tile compute.
  * Out-projection lags one tile and stages per-512-col chunks (3 DVE /
    1 ACT copies).  NOTE measured dead ends: DMA straight from PSUM is
    not allowed (dma_start src must be SBUF); interleaving the staging
    copies between y-chain heads + grouped 4-slab weight DMAs DID fill
    the PE gaps (43us idle vs 59us) but made every matmul ~20% slower
    from SBUF port contention - net 274us vs 256us.  Less concurrency
    won here.
  * fp8 DoubleRow was tried (v4) and reverted: with --enable-ldw-opt=false
    every matmul pays its own LDWEIGHTS, so 3x more 256-col instructions
    lose to bf16's 512-col streams (314us vs 256us).
  * tensor_tensor_reduce / Pool-engine ops crash this device - avoided.

Attention computes TRANSPOSED scores ST[j, i] = k_j . q_i directly
(stationary = kT block, moving = all 4 heads' scaled qT), so the exp
output IS P^T in SBUF (no PE transpose matmuls for P).  Row sums come
from a ones-column appended to V (PV matmul streams 129 cols; col 128
accumulates sum_j P[i, j]).  Softmax normalization is applied
per-partition while copying the PV accumulator out of PSUM; an identity-
transpose per (tile, head) yields yT for the out-projection.
"""

import numpy as np
import ml_dtypes

import concourse.bass as bass
import concourse.bacc as bacc
import concourse.tile as tile
from concourse import mybir
from concourse import bass_utils

BF = mybir.dt.bfloat16
F32 = mybir.dt.float32
AF = mybir.ActivationFunctionType
ALU = mybir.AluOpType

B = 2
T = 2048
E = 2048
D = 128          # head dim
HQ = 4           # q heads per core (one kv group)
NKV = 4
NT = T // 128    # 16 t-tiles
NE = E // 128    # 16 e-tiles
W = 512          # sliding window
NJB = W // 128   # history blocks
EPS = float(np.finfo(np.float32).eps)


def _bcast_mid(ap, n):
    """Insert a step-0 dim after the partition dim: [p, w] -> [p, n, w]."""
    return bass.AP(tensor=ap.tensor, offset=ap.offset,
                   ap=[ap.ap[0], [0, n], *ap.ap[1:]])


def _half_swap(ap2d, nmid):
    """[p, nmid*128] -> [p, nmid, 2, 64] view with the 64-halves swapped."""
    return bass.AP(tensor=ap2d.tensor, offset=ap2d.offset + 64,
                   ap=[ap2d.ap[0], [128, nmid], [-64, 2], [1, 64]])


def _body(tc, io):
    nc = tc.nc
    xT, wq, wkvg, wo, ve2, cosd, sind, masks, ident, out = (
        io["xT"], io["wq"], io["wkvg"], io["wo"], io["ve2"], io["cos"],
        io["sin"], io["masks"], io["ident"], io["out"])

    with (
        tc.tile_pool(name="const", bufs=1) as cpool,
        tc.tile_pool(name="state", bufs=1) as state,
    ):
        ident_sb = cpool.tile([128, 128], BF, tag="ident")
        nc.sync.dma_start(ident_sb, ident)
        cos_sb = cpool.tile([128, NT, 128], BF, tag="cos")
        sin_sb = cpool.tile([128, NT, 128], BF, tag="sin")
        ve_sb = cpool.tile([128, NT, D], BF, tag="ve")
        cosr = cosd.rearrange("(t p) h -> p t h", p=128)
        sinr = sind.rearrange("(t p) h -> p t h", p=128)
        ver = ve2.rearrange("(t p) d -> p t d", p=128)
        nc.sync.dma_start(cos_sb[:, 0:2, :], cosr[:, 0:2, :])
        nc.sync.dma_start(sin_sb[:, 0:2, :], sinr[:, 0:2, :])
        nc.sync.dma_start(ve_sb[:, 0:2, :], ver[:, 0:2, :])

        wqkvg_sb = cpool.tile([128, NE, 768], BF, tag="wqkvg")
        wqd = wq.rearrange("(e p) f -> p e f", p=128)
        wkvgd = wkvg.rearrange("(e p) f -> p e f", p=128)
        xTr = xT.rearrange("(e p) t -> p e t", p=128)
        with tc.tile_pool(name="xp", bufs=1) as xp:
            xT_sb = xp.tile([128, NE, T], BF, tag="xT")
            # weights + the first 256 t-cols of x, per contraction slab:
            # tile-0/1 projections start as soon as slab 0 lands.
            for e in range(NE):
                nc.sync.dma_start(wqkvg_sb[:, e, 0:512], wqd[:, e])
                nc.sync.dma_start(wqkvg_sb[:, e, 512:768], wkvgd[:, e])
                nc.sync.dma_start(xT_sb[:, e, 0:256], xTr[:, e, 0:256])
            # x slices for tiles 2-5 BEFORE the small-tensor remainder:
            # the measured 6.8us PE stall at t~44us was phase A waiting
            # for slice tb=1 behind the cos/sin/ve bytes.
            for tb in (1, 2):
                cs = slice(tb * 256, (tb + 1) * 256)
                nc.sync.dma_start(xT_sb[:, :, cs], xTr[:, :, cs])
            nc.sync.dma_start(cos_sb[:, 2:, :], cosr[:, 2:, :])
            nc.sync.dma_start(sin_sb[:, 2:, :], sinr[:, 2:, :])
            nc.sync.dma_start(ve_sb[:, 2:, :], ver[:, 2:, :])
            for tb in range(3, T // 256):
                cs = slice(tb * 256, (tb + 1) * 256)
                nc.sync.dma_start(xT_sb[:, :, cs], xTr[:, :, cs])
            masks_sb = cpool.tile([128, 2, 128], BF, tag="masks")
            nc.sync.dma_start(masks_sb, masks.rearrange("m p j -> p m j"))
            wo_sb = cpool.tile([128, HQ, E], BF, tag="wo")
            nc.sync.dma_start(wo_sb, wo.rearrange("(h p) e -> p h e", p=128))

            # ---- whole-sequence state ------------------------------------
            qkT_all = state.tile([128, NT, HQ, 128], BF, tag="qkT")
            kT_all = state.tile([128, NT, 128], BF, tag="kT")
            v_all = state.tile([128, NT, 129], BF, tag="v")
            yT_all = state.tile([128, NT, HQ, 128], BF, tag="yT")

            # ones column for PV row sums (independent of v_all's data cols)
            nc.vector.tensor_scalar(
                v_all[:, :, 128:129].rearrange("p a b -> p (a b)"),
                ident_sb[:, 0:NT], 0.0, 1.0, op0=ALU.mult, op1=ALU.add)

            # ============ PHASE A: proj + rope + rms + transposes =========
            with (
                tc.tile_pool(name="workA", bufs=3) as work,
                tc.tile_pool(name="psA_q", bufs=4, space="PSUM") as psA_q,
                tc.tile_pool(name="psA_kvg", bufs=2, space="PSUM") as psA_kvg,
                tc.tile_pool(name="psA_tr", bufs=1, space="PSUM") as psA_tr,
            ):
                def _emit_transposes(ptt, qros_p, krms_p):
                    trp = psA_tr.tile([128, HQ + 1, 128], F32, tag="trp")
                    for h in range(HQ):
                        nc.tensor.matmul(trp[:, h, :], qros_p[:, h, :],
                                         ident_sb, start=True, stop=True)
                    nc.tensor.matmul(trp[:, HQ, :], krms_p, ident_sb,
                                     start=True, stop=True)
                    # qT copy on ACT (Copy is a filler in every table set)
                    nc.scalar.activation(
                        qkT_all[:, ptt, :, :].rearrange("p a b -> p (a b)"),
                        trp[:, 0:HQ, :].rearrange("p a b -> p (a b)"),
                        AF.Copy)
                    nc.vector.tensor_copy(kT_all[:, ptt, :], trp[:, HQ, :])

                lagq = []

                def _proj_post(tt, psq, pskvg):
                    ts = slice(tt * 128, (tt + 1) * 128)
                    # v = v_proj + gated ve (gate pre-applied on host)
                    nc.vector.tensor_add(v_all[:, tt, 0:128],
                                         pskvg[:, 128:256], ve_sb[:, tt, :])

                    # cast projections to bf16, then RoPE in fast DVE modes
                    qb = work.tile([128, HQ * D], BF, tag="qb")
                    nc.vector.tensor_copy(qb, psq)
                    kb = work.tile([128, D], BF, tag="kb")
                    nc.vector.tensor_copy(kb, pskvg[:, 0:128])

                    cosq = _bcast_mid(cos_sb[:, tt, :], HQ)
                    sinq = _bcast_mid(sin_sb[:, tt, :], HQ)
                    qro = work.tile([128, HQ, D], BF, tag="qro")
                    rb = work.tile([128, HQ, D], BF, tag="rb")
                    nc.vector.tensor_mul(
                        qro, qb.rearrange("p (h d) -> p h d", h=HQ), cosq)
                    nc.vector.tensor_mul(
                        rb.rearrange("p h (s x) -> p h s x", s=2),
                        _half_swap(qb[:], HQ),
                        sinq.rearrange("p h (s x) -> p h s x", s=2))
                    nc.vector.tensor_add(qro, qro, rb)

                    kro = work.tile([128, D], BF, tag="kro")
                    kb2 = work.tile([128, D], BF, tag="kb2")
                    nc.vector.tensor_mul(kro, kb, cos_sb[:, tt, :])
                    nc.vector.tensor_mul(
                        kb2.rearrange("p (s x) -> p s x", s=2),
                        _half_swap(kb[:], 1),
                        sin_sb[:, tt, :].rearrange("p (s x) -> p s x", s=2))
                    nc.vector.tensor_add(kro, kro, kb2)

                    # RMS sums of squares (ACT Square + accum)
                    sq = work.tile([128, (HQ + 1) * D], BF, tag="sq")
                    qkss = work.tile([128, HQ + 1], F32, tag="qkss")
                    for h in range(HQ):
                        nc.scalar.activation(sq[:, h * D:(h + 1) * D],
                                             qro[:, h, :], AF.Square,
                                             accum_out=qkss[:, h:h + 1])
                    nc.scalar.activation(sq[:, HQ * D:], kro, AF.Square,
                                         accum_out=qkss[:, HQ:HQ + 1])
                    # rsqrt = DVE reciprocal(ACT Sqrt); the k column folds
                    # the sqrt(D) score scale via a 1/D pre-multiply.
                    tmp5 = work.tile([128, HQ + 1], F32, tag="tmp5")
                    nc.vector.tensor_scalar_add(tmp5[:, 0:HQ],
                                                qkss[:, 0:HQ],
                                                float(128.0 * EPS))
                    nc.vector.tensor_scalar(tmp5[:, HQ:HQ + 1],
                                            qkss[:, HQ:HQ + 1],
                                            float(1.0 / 128.0), EPS,
                                            op0=ALU.mult, op1=ALU.add)
                    rt5 = work.tile([128, HQ + 1], F32, tag="rt5")
                    nc.scalar.activation(rt5, tmp5, AF.Sqrt)
                    sc5 = work.tile([128, HQ + 1], F32, tag="sc5")
                    nc.vector.reciprocal(sc5, rt5)

                    # krms = kro * sc5_k ; qros_h = qro_h * sc5_h (DVE)
                    krms = work.tile([128, D], BF, tag="krms")
                    nc.vector.tensor_scalar_mul(krms, kro,
                                                sc5[:, HQ:HQ + 1])
                    qros = work.tile([128, HQ, D], BF, tag="qros")
                    for h in range(HQ):
                        nc.vector.tensor_scalar_mul(qros[:, h, :],
                                                    qro[:, h, :],
                                                    sc5[:, h:h + 1])

                    # transposes run TWO tiles lagged so the PE never
                    # waits on the DVE/ACT chain
                    lagq.append((tt, qros, krms))
                    if len(lagq) > 2:
                        _emit_transposes(*lagq.pop(0))

                # projections run in tile PAIRS with the q matmuls
                # interleaved per contraction slab: during the initial
                # weight stream each arriving slab unlocks ~1.3us of PE
                # work instead of ~0.65us (tile-0-only), halving the
                # measured per-slab chase gaps at t~15-44us.
                for tp in range(0, NT, 2):
                    ts0 = slice(tp * 128, (tp + 1) * 128)
                    ts1 = slice((tp + 1) * 128, (tp + 2) * 128)
                    psq0 = psA_q.tile([128, 512], F32, tag="psq")
                    psq1 = psA_q.tile([128, 512], F32, tag="psq")
                    kvg0 = psA_kvg.tile([128, 256], F32, tag="pskvg")
                    kvg1 = psA_kvg.tile([128, 256], F32, tag="pskvg")
                    for e in range(NE):
                        nc.tensor.matmul(psq0, xT_sb[:, e, ts0],
                                         wqkvg_sb[:, e, 0:512],
                                         start=(e == 0), stop=(e == NE - 1))
                        nc.tensor.matmul(psq1, xT_sb[:, e, ts1],
                                         wqkvg_sb[:, e, 0:512],
                                         start=(e == 0), stop=(e == NE - 1))
                    for e in range(NE):
                        nc.tensor.matmul(kvg0, xT_sb[:, e, ts0],
                                         wqkvg_sb[:, e, 512:768],
                                         start=(e == 0), stop=(e == NE - 1))
                        nc.tensor.matmul(kvg1, xT_sb[:, e, ts1],
                                         wqkvg_sb[:, e, 512:768],
                                         start=(e == 0), stop=(e == NE - 1))
                    _proj_post(tp, psq0, kvg0)
                    _proj_post(tp + 1, psq1, kvg1)
                for item in lagq:
                    _emit_transposes(*item)

        # ============= PHASE B: attention ============================
        with (
            tc.tile_pool(name="attn", bufs=3) as attn,
            # outc MUST stay at 2: bufs=3 measured 284us and bufs=4 281us
            # (vs 252us) - deeper staging lets the out-proj copies run
            # ahead and the added engine concurrency slows every matmul
            # stream via SBUF port contention.
            tc.tile_pool(name="outc", bufs=2) as outc,
            tc.tile_pool(name="psB", bufs=3, space="PSUM") as psB,
            tc.tile_pool(name="psY", bufs=2, space="PSUM") as psY,
            tc.tile_pool(name="psC", bufs=3, space="PSUM") as psC,
        ):
            def _emit_outproj(ptt):
                # four 512-col chunks, each DMA'd as soon as it lands
                pts = slice(ptt * 128, (ptt + 1) * 128)
                for ec in range(4):
                    ops = psC.tile([128, 512], F32, tag="ops")
                    for h in range(HQ):
                        nc.tensor.matmul(
                            ops, yT_all[:, ptt, h, :],
                            wo_sb[:, h, ec * 512:(ec + 1) * 512],
                            start=(h == 0), stop=(h == HQ - 1))
                    osb = outc.tile([128, 512], BF, tag="osb")
                    if ec == 3:
                        nc.scalar.activation(osb, ops, AF.Copy)
                    else:
                        nc.vector.tensor_copy(osb, ops)
                    nc.sync.dma_start(out[pts, ec * 512:(ec + 1) * 512],
                                      osb)

            for tt in range(NT):
                njb = min(tt, NJB) + 1
                jb0 = tt - (njb - 1)
                nhist = njb - 1
                # scores for ALL 4 heads per key block: one 512-col
                # matmul (stationary kT block, moving all heads' qT),
                # exp'd per block.  Masks are applied POST-exp as
                # multiplicative 0/1 bf16 on the DVE.
                pexp = attn.tile([128, NJB + 3, HQ, 128], BF, tag="pexp")
                slot = list(range(njb))
                for jb in range(njb):
                    st = psB.tile([128, HQ, 128], F32, tag="st")
                    nc.tensor.matmul(
                        st.rearrange("p h i -> p (h i)"),
                        kT_all[:, jb0 + jb, :],
                        qkT_all[:, tt, :, :].rearrange(
                            "p h i -> p (h i)"),
                        start=True, stop=True)
                    nc.scalar.activation(
                        pexp[:, jb, :, :].rearrange("p h i -> p (h i)"),
                        st.rearrange("p h i -> p (h i)"), AF.Exp)
                    if jb == nhist:
                        nc.vector.tensor_mul(
                            pexp[:, NJB + 2, :, :], pexp[:, jb, :, :],
                            _bcast_mid(masks_sb[:, 1, :], HQ))
                        slot[jb] = NJB + 2
                    elif jb == 0 and tt >= NJB:
                        nc.vector.tensor_mul(
                            pexp[:, NJB + 1, :, :], pexp[:, jb, :, :],
                            _bcast_mid(masks_sb[:, 0, :], HQ))
                        slot[jb] = NJB + 1

                for h in range(HQ):
                    # PV with ones column: col 128 = row sums.  The
                    # y tile also hosts the yT transpose result in cols
                    # 129:257 so both fit one PSUM bank.
                    y_ps = psY.tile([128, 257], F32, tag="y")
                    for jb in range(njb):
                        nc.tensor.matmul(y_ps[:, 0:129],
                                         pexp[:, slot[jb], h, :],
                                         v_all[:, jb0 + jb, :],
                                         start=(jb == 0),
                                         stop=(jb == njb - 1))
                    rsum = attn.tile([128, 1], F32, tag="rsum")
                    nc.vector.reciprocal(rsum, y_ps[:, 128:129])
                    y_sb = attn.tile([128, 128], BF, tag="ysb")
                    if h % 2 == 0:
                        nc.vector.tensor_scalar_mul(y_sb,
                                                    y_ps[:, 0:128],
                                                    rsum)
                    else:
                        nc.scalar.activation(y_sb, y_ps[:, 0:128],
                                             AF.Copy, scale=rsum)
                    # transpose y -> yT
                    nc.tensor.matmul(y_ps[:, 129:257], y_sb, ident_sb,
                                     start=True, stop=True)
                    nc.vector.tensor_copy(yT_all[:, tt, h, :],
                                          y_ps[:, 129:257])

                # out-projection runs one tile LAGGED so the PE
                # never stalls on this tile's y normalization chain
                if tt > 0:
                    _emit_outproj(tt - 1)
            _emit_outproj(NT - 1)


def build_nc(stage=99):
    nc = bacc.Bacc("TRN2", target_bir_lowering=False, debug=False,
                   num_devices=8)
    io = {
        "xT": nc.dram_tensor("xT", [E, T], BF, kind="ExternalInput").ap(),
        "wq": nc.dram_tensor("wq", [E, HQ * D], BF, kind="ExternalInput").ap(),
        "wkvg": nc.dram_tensor("wkvg", [E, 256], BF, kind="ExternalInput").ap(),
        "wo": nc.dram_tensor("wo", [HQ * D, E], BF, kind="ExternalInput").ap(),
        "ve2": nc.dram_tensor("ve2", [T, D], BF, kind="ExternalInput").ap(),
        "cos": nc.dram_tensor("cos", [T, 128], BF, kind="ExternalInput").ap(),
        "sin": nc.dram_tensor("sin", [T, 128], BF, kind="ExternalInput").ap(),
        "masks": nc.dram_tensor("masks", [2, 128, 128], BF,
                                kind="ExternalInput").ap(),
        "ident": nc.dram_tensor("ident", [128, 128], BF,
                                kind="ExternalInput").ap(),
        "out": nc.dram_tensor("out", [T, E], BF, kind="ExternalOutput").ap(),
    }
    with tile.TileContext(nc) as tc:
        _body(tc, io)
    nc.compile()
    return nc


_NC = None


def _get_nc():
    global _NC
    if _NC is None:
        _NC = build_nc()
    return _NC


def _prep_in_maps(x, ve, cos, sin, wq, wk, wv, wo, wgate):
    x = np.asarray(x, dtype=np.float32)
    ve = np.asarray(ve, dtype=np.float32)
    cos1 = np.asarray(cos, np.float32).reshape(T, 64)
    sin1 = np.asarray(sin, np.float32).reshape(T, 64)
    cos2 = np.ascontiguousarray(
        np.concatenate([cos1, cos1], axis=1)).astype(ml_dtypes.bfloat16)
    sin2 = np.ascontiguousarray(
        np.concatenate([sin1, -sin1], axis=1)).astype(ml_dtypes.bfloat16)
    ii = np.arange(128)
    masks = np.ones((2, 128, 128), np.float32)
    # multiplicative post-exp masks, transposed for the P^T[j, i] layout:
    # window-edge block: kill keys j <= i ; causal diag block: kill j > i
    masks[0][ii[:, None] <= ii[None, :]] = 0.0
    masks[1][ii[:, None] > ii[None, :]] = 0.0
    masks = np.ascontiguousarray(masks).astype(ml_dtypes.bfloat16)
    ident = np.eye(128, dtype=ml_dtypes.bfloat16)

    xT_b = [np.ascontiguousarray(x[b].T).astype(ml_dtypes.bfloat16)
            for b in range(B)]
    in_maps = []
    for c in range(8):
        b, g = divmod(c, NKV)
        wq_c = np.ascontiguousarray(
            wq[g * 512:(g + 1) * 512, :].T).astype(ml_dtypes.bfloat16)
        wk_c = wk[g * 128:(g + 1) * 128, :].T
        wv_c = wv[g * 128:(g + 1) * 128, :].T
        wkvg_c = np.ascontiguousarray(
            np.concatenate([wk_c, wv_c], axis=1)).astype(ml_dtypes.bfloat16)
        wo_c = np.ascontiguousarray(
            wo[:, g * 512:(g + 1) * 512].T).astype(ml_dtypes.bfloat16)
        # value-embedding gate folded on the host (tiny 32-ch matmul)
        logit = x[b, :, :32] @ wgate[g]
        gate = 2.0 / (1.0 + np.exp(-logit))
        ve2_c = np.ascontiguousarray(
            gate[:, None] * ve[b, :, g * 128:(g + 1) * 128]).astype(
                ml_dtypes.bfloat16)
        in_maps.append({
            "xT": xT_b[b], "wq": wq_c, "wkvg": wkvg_c, "wo": wo_c,
            "ve2": ve2_c, "cos": cos2, "sin": sin2, "masks": masks,
            "ident": ident,
        })
    return in_maps


def kernel(x, ve, cos, sin, wq, wk, wv, wo, wgate, window_size=512,
           _trace=False):
    assert int(window_size) == W, f"kernel hardcodes window {W}"
    wq = np.asarray(wq, np.float32)
    wk = np.asarray(wk, np.float32)
    wv = np.asarray(wv, np.float32)
    wo = np.asarray(wo, np.float32)
    wgate = np.asarray(wgate, np.float32)
    in_maps = _prep_in_maps(x, ve, cos, sin, wq, wk, wv, wo, wgate)
    nc = _get_nc()
    res = bass_utils.run_bass_kernel_spmd(
        nc, in_maps, core_ids=list(range(8)), trace=_trace)
    out = np.empty((B, T, E), np.float32)
    for b in range(B):
        acc = res.results[b * NKV]["out"].astype(np.float32).copy()
        for g in range(1, NKV):
            acc += res.results[b * NKV + g]["out"]
        out[b] = acc
    if _trace:
        kernel.last_results = res
    return out


# revision 26
# speedup vs baseline: 1.2159x; 1.0683x over previous
"""Trainium2 Bass kernel for NanochatAttention (sliding-window GQA attention).

Sharding: 8 cores = (batch b in {0,1}) x (kv-group g in {0..3}).
Each core handles one batch's full sequence for one KV head and its 4 Q heads:
projections, RoPE + QK RMS-norm, value-embedding gate, 512-window causal
attention, and the row-parallel out-projection slice -> partial [T, E] output.
Host sums the 4 partials per batch at unshard time.

v5 highlights (evidence-driven, see trace history):
  * RMS rsqrt = ACT Sqrt + DVE reciprocal.  Square/Sqrt/Copy share ONE
    activation-table set, Exp (phase B) is the only other set -> 2 table
    loads total.  (v2's Ln+Exp rsqrt alternated two sets: 33 loads,
    ~2.6us/tile of Scalar-engine stall that backed up the PE.)
  * DMA: few big issues (each sync dma_start costs ~0.6us of sync-engine
    time).  Weights + the first 256 t-cols of x go in 4-slab groups so
    tile-0's projection chain starts as soon as group 0 lands; x remainder
    streams in 256-col slices ahead of the per-tile compute.
  * Out-projection lags one tile and stages per-512-col chunks (3 DVE /
    1 ACT copies).  NOTE measured dead ends: DMA straight from PSUM is
    not allowed (dma_start src must be SBUF); interleaving the staging
    copies between y-chain heads + grouped 4-slab weight DMAs DID fill
    the PE gaps (43us idle vs 59us) but made every matmul ~20% slower
    from SBUF port contention - net 274us vs 256us.  Less concurrency
    won here.
  * fp8 DoubleRow was tried (v4) and reverted: with --enable-ldw-opt=false
    every matmul pays its own LDWEIGHTS, so 3x more 256-col instructions
    lose to bf16's 512-col streams (314us vs 256us).
  * tensor_tensor_reduce / Pool-engine ops crash this device - avoided.

Attention computes TRANSPOSED scores ST[j, i] = k_j . q_i directly
(stationary = kT block, moving = all 4 heads' scaled qT), so the exp
output IS P^T in SBUF (no PE transpose matmuls for P).  Row sums come
from a ones-column appended to V (PV matmul streams 129 cols; col 128
accumulates sum_j P[i, j]).  Softmax normalization is applied
per-partition while copying the PV accumulator out of PSUM; an identity-
transpose per (tile, head) yields yT for the out-projection.
"""

import numpy as np
import ml_dtypes

import concourse.bass as bass
import concourse.bacc as bacc
import concourse.tile as tile
from concourse import mybir
from concourse import bass_utils

BF = mybir.dt.bfloat16
F32 = mybir.dt.float32
AF = mybir.ActivationFunctionType
ALU = mybir.AluOpType

B = 2
T = 2048
E = 2048
D = 128          # head dim
HQ = 4           # q heads per core (one kv group)
NKV = 4
NT = T // 128    # 16 t-tiles
NE = E // 128    # 16 e-tiles
W = 512          # sliding window
NJB = W // 128   # history blocks
EPS = float(np.finfo(np.float32).eps)


def _bcast_mid(ap, n):
    """Insert a step-0 dim after the partition dim: [p, w] -> [p, n, w]."""
    return bass.AP(tensor=ap.tensor, offset=ap.offset,
                   ap=[ap.ap[0], [0, n], *ap.ap[1:]])


def _half_swap(ap2d, nmid):
    """[p, nmid*128] -> [p, nmid, 2, 64] view with the 64-halves swapped."""
    return bass.AP(tensor=ap2d.tensor, offset=ap2d.offset + 64,
                   ap=[ap2d.ap[0], [128, nmid], [-64, 2], [1, 64]])


def _body(tc, io):
    nc = tc.nc
    xT, wq, wkvg, wo, ve2, cosd, sind, masks, ident, out = (
        io["xT"], io["wq"], io["wkvg"], io["wo"], io["ve2"], io["cos"],
        io["sin"], io["masks"], io["ident"], io["out"])

    with (
        tc.tile_pool(name="const", bufs=1) as cpool,
        tc.tile_pool(name="state", bufs=1) as state,
    ):
        ident_sb = cpool.tile([128, 128], BF, tag="ident")
        nc.sync.dma_start(ident_sb, ident)
        cos_sb = cpool.tile([128, NT, 128], BF, tag="cos")
        sin_sb = cpool.tile([128, NT, 128], BF, tag="sin")
        ve_sb = cpool.tile([128, NT, D], BF, tag="ve")
        cosr = cosd.rearrange("(t p) h -> p t h", p=128)
        sinr = sind.rearrange("(t p) h -> p t h", p=128)
        ver = ve2.rearrange("(t p) d -> p t d", p=128)
        nc.sync.dma_start(cos_sb[:, 0:2, :], cosr[:, 0:2, :])
        nc.sync.dma_start(sin_sb[:, 0:2, :], sinr[:, 0:2, :])
        nc.sync.dma_start(ve_sb[:, 0:2, :], ver[:, 0:2, :])

        wqkvg_sb = cpool.tile([128, NE, 768], BF, tag="wqkvg")
        wqd = wq.rearrange("(e p) f -> p e f", p=128)
        wkvgd = wkvg.rearrange("(e p) f -> p e f", p=128)
        xTr = xT.rearrange("(e p) t -> p e t", p=128)
        with tc.tile_pool(name="xp", bufs=1) as xp:
            xT_sb = xp.tile([128, NE, T], BF, tag="xT")
            # weights + the first 256 t-cols of x in 4-slab groups: the
            # measured ~2us/slab arrival cadence was ISSUE-limited (3
            # dma_starts x 0.6us sync time per slab), so batching 4 slabs
            # per issue feeds the paired projection stream without gaps.
            for eg in range(0, NE, 4):
                es = slice(eg, eg + 4)
                nc.sync.dma_start(wqkvg_sb[:, es, 0:512], wqd[:, es])
                nc.sync.dma_start(wqkvg_sb[:, es, 512:768], wkvgd[:, es])
                nc.sync.dma_start(xT_sb[:, es, 0:256], xTr[:, es, 0:256])
            # x slices for tiles 2-5 BEFORE the small-tensor remainder:
            # the measured 6.8us PE stall at t~44us was phase A waiting
            # for slice tb=1 behind the cos/sin/ve bytes.
            for tb in (1, 2):
                cs = slice(tb * 256, (tb + 1) * 256)
                nc.sync.dma_start(xT_sb[:, :, cs], xTr[:, :, cs])
            nc.sync.dma_start(cos_sb[:, 2:, :], cosr[:, 2:, :])
            nc.sync.dma_start(sin_sb[:, 2:, :], sinr[:, 2:, :])
            nc.sync.dma_start(ve_sb[:, 2:, :], ver[:, 2:, :])
            for tb in range(3, T // 256):
                cs = slice(tb * 256, (tb + 1) * 256)
                nc.sync.dma_start(xT_sb[:, :, cs], xTr[:, :, cs])
            masks_sb = cpool.tile([128, 2, 128], BF, tag="masks")
            nc.sync.dma_start(masks_sb, masks.rearrange("m p j -> p m j"))
            wo_sb = cpool.tile([128, HQ, E], BF, tag="wo")
            nc.sync.dma_start(wo_sb, wo.rearrange("(h p) e -> p h e", p=128))

            # ---- whole-sequence state ------------------------------------
            qkT_all = state.tile([128, NT, HQ, 128], BF, tag="qkT")
            kT_all = state.tile([128, NT, 128], BF, tag="kT")
            v_all = state.tile([128, NT, 129], BF, tag="v")
            yT_all = state.tile([128, NT, HQ, 128], BF, tag="yT")

            # ones column for PV row sums (independent of v_all's data cols)
            nc.vector.tensor_scalar(
                v_all[:, :, 128:129].rearrange("p a b -> p (a b)"),
                ident_sb[:, 0:NT], 0.0, 1.0, op0=ALU.mult, op1=ALU.add)

            # ============ PHASE A: proj + rope + rms + transposes =========
            with (
                tc.tile_pool(name="workA", bufs=3) as work,
                tc.tile_pool(name="psA_q", bufs=4, space="PSUM") as psA_q,
                tc.tile_pool(name="psA_kvg", bufs=2, space="PSUM") as psA_kvg,
                tc.tile_pool(name="psA_tr", bufs=1, space="PSUM") as psA_tr,
            ):
                def _emit_transposes(ptt, qros_p, krms_p):
                    trp = psA_tr.tile([128, HQ + 1, 128], F32, tag="trp")
                    for h in range(HQ):
                        nc.tensor.matmul(trp[:, h, :], qros_p[:, h, :],
                                         ident_sb, start=True, stop=True)
                    nc.tensor.matmul(trp[:, HQ, :], krms_p, ident_sb,
                                     start=True, stop=True)
                    # qT copy on ACT (Copy is a filler in every table set)
                    nc.scalar.activation(
                        qkT_all[:, ptt, :, :].rearrange("p a b -> p (a b)"),
                        trp[:, 0:HQ, :].rearrange("p a b -> p (a b)"),
                        AF.Copy)
                    nc.vector.tensor_copy(kT_all[:, ptt, :], trp[:, HQ, :])

                lagq = []

                def _proj_post(tt, psq, pskvg):
                    ts = slice(tt * 128, (tt + 1) * 128)
                    # v = v_proj + gated ve (gate pre-applied on host)
                    nc.vector.tensor_add(v_all[:, tt, 0:128],
                                         pskvg[:, 128:256], ve_sb[:, tt, :])

                    # cast projections to bf16, then RoPE in fast DVE modes
                    qb = work.tile([128, HQ * D], BF, tag="qb")
                    nc.vector.tensor_copy(qb, psq)
                    kb = work.tile([128, D], BF, tag="kb")
                    nc.vector.tensor_copy(kb, pskvg[:, 0:128])

                    cosq = _bcast_mid(cos_sb[:, tt, :], HQ)
                    sinq = _bcast_mid(sin_sb[:, tt, :], HQ)
                    qro = work.tile([128, HQ, D], BF, tag="qro")
                    rb = work.tile([128, HQ, D], BF, tag="rb")
                    nc.vector.tensor_mul(
                        qro, qb.rearrange("p (h d) -> p h d", h=HQ), cosq)
                    nc.vector.tensor_mul(
                        rb.rearrange("p h (s x) -> p h s x", s=2),
                        _half_swap(qb[:], HQ),
                        sinq.rearrange("p h (s x) -> p h s x", s=2))
                    nc.vector.tensor_add(qro, qro, rb)

                    kro = work.tile([128, D], BF, tag="kro")
                    kb2 = work.tile([128, D], BF, tag="kb2")
                    nc.vector.tensor_mul(kro, kb, cos_sb[:, tt, :])
                    nc.vector.tensor_mul(
                        kb2.rearrange("p (s x) -> p s x", s=2),
                        _half_swap(kb[:], 1),
                        sin_sb[:, tt, :].rearrange("p (s x) -> p s x", s=2))
                    nc.vector.tensor_add(kro, kro, kb2)

                    # RMS sums of squares (ACT Square + accum)
                    sq = work.tile([128, (HQ + 1) * D], BF, tag="sq")
                    qkss = work.tile([128, HQ + 1], F32, tag="qkss")
                    for h in range(HQ):
                        nc.scalar.activation(sq[:, h * D:(h + 1) * D],
                                             qro[:, h, :], AF.Square,
                                             accum_out=qkss[:, h:h + 1])
                    nc.scalar.activation(sq[:, HQ * D:], kro, AF.Square,
                                         accum_out=qkss[:, HQ:HQ + 1])
                    # rsqrt = DVE reciprocal(ACT Sqrt); the k column folds
                    # the sqrt(D) score scale via a 1/D pre-multiply.
                    tmp5 = work.tile([128, HQ + 1], F32, tag="tmp5")
                    nc.vector.tensor_scalar_add(tmp5[:, 0:HQ],
                                                qkss[:, 0:HQ],
                                                float(128.0 * EPS))
                    nc.vector.tensor_scalar(tmp5[:, HQ:HQ + 1],
                                            qkss[:, HQ:HQ + 1],
                                            float(1.0 / 128.0), EPS,
                                            op0=ALU.mult, op1=ALU.add)
                    rt5 = work.tile([128, HQ + 1], F32, tag="rt5")
                    nc.scalar.activation(rt5, tmp5, AF.Sqrt)
                    sc5 = work.tile([128, HQ + 1], F32, tag="sc5")
                    nc.vector.reciprocal(sc5, rt5)

                    # krms = kro * sc5_k ; qros_h = qro_h * sc5_h (DVE)
                    krms = work.tile([128, D], BF, tag="krms")
                    nc.vector.tensor_scalar_mul(krms, kro,
                                                sc5[:, HQ:HQ + 1])
                    qros = work.tile([128, HQ, D], BF, tag="qros")
                    for h in range(HQ):
                        nc.vector.tensor_scalar_mul(qros[:, h, :],
                                                    qro[:, h, :],
                                                    sc5[:, h:h + 1])

                    # transposes run TWO tiles lagged so the PE never
                    # waits on the DVE/ACT chain
                    lagq.append((tt, qros, krms))
                    if len(lagq) > 2:
                        _emit_transposes(*lagq.pop(0))

                # projections run in tile PAIRS with the q matmuls
                # interleaved per contraction slab: during the initial
                # weight stream each arriving slab unlocks ~1.3us of PE
                # work instead of ~0.65us (tile-0-only), halving the
                # measured per-slab chase gaps at t~15-44us.
                for tp in range(0, NT, 2):
                    ts0 = slice(tp * 128, (tp + 1) * 128)
                    ts1 = slice((tp + 1) * 128, (tp + 2) * 128)
                    psq0 = psA_q.tile([128, 512], F32, tag="psq")
                    psq1 = psA_q.tile([128, 512], F32, tag="psq")
                    kvg0 = psA_kvg.tile([128, 256], F32, tag="pskvg")
                    kvg1 = psA_kvg.tile([128, 256], F32, tag="pskvg")
                    for e in range(NE):
                        nc.tensor.matmul(psq0, xT_sb[:, e, ts0],
                                         wqkvg_sb[:, e, 0:512],
                                         start=(e == 0), stop=(e == NE - 1))
                        nc.tensor.matmul(psq1, xT_sb[:, e, ts1],
                                         wqkvg_sb[:, e, 0:512],
                                         start=(e == 0), stop=(e == NE - 1))
                    for e in range(NE):
                        nc.tensor.matmul(kvg0, xT_sb[:, e, ts0],
                                         wqkvg_sb[:, e, 512:768],
                                         start=(e == 0), stop=(e == NE - 1))
                        nc.tensor.matmul(kvg1, xT_sb[:, e, ts1],
                                         wqkvg_sb[:, e, 512:768],
                                         start=(e == 0), stop=(e == NE - 1))
                    _proj_post(tp, psq0, kvg0)
                    _proj_post(tp + 1, psq1, kvg1)
                for item in lagq:
                    _emit_transposes(*item)

        # ============= PHASE B: attention ============================
        with (
            tc.tile_pool(name="attn", bufs=3) as attn,
            # outc MUST stay at 2: bufs=3 measured 284us and bufs=4 281us
            # (vs 252us) - deeper staging lets the out-proj copies run
            # ahead and the added engine concurrency slows every matmul
            # stream via SBUF port contention.
            tc.tile_pool(name="outc", bufs=2) as outc,
            tc.tile_pool(name="psB", bufs=3, space="PSUM") as psB,
            tc.tile_pool(name="psY", bufs=2, space="PSUM") as psY,
            tc.tile_pool(name="psC", bufs=3, space="PSUM") as psC,
        ):
            def _emit_outproj(ptt):
                # four 512-col chunks, each DMA'd as soon as it lands
                pts = slice(ptt * 128, (ptt + 1) * 128)
                for ec in range(4):
                    ops = psC.tile([128, 512], F32, tag="ops")
                    for h in range(HQ):
                        nc.tensor.matmul(
                            ops, yT_all[:, ptt, h, :],
                            wo_sb[:, h, ec * 512:(ec + 1) * 512],
                            start=(h == 0), stop=(h == HQ - 1))
                    osb = outc.tile([128, 512], BF, tag="osb")
                    if ec == 3:
                        nc.scalar.activation(osb, ops, AF.Copy)
                    else:
                        nc.vector.tensor_copy(osb, ops)
                    nc.sync.dma_start(out[pts, ec * 512:(ec + 1) * 512],
                                      osb)

            for tt in range(NT):
                njb = min(tt, NJB) + 1
                jb0 = tt - (njb - 1)
                nhist = njb - 1
                # scores for ALL 4 heads per key block: one 512-col
                # matmul (stationary kT block, moving all heads' qT),
                # exp'd per block.  Masks are applied POST-exp as
                # multiplicative 0/1 bf16 on the DVE.
                pexp = attn.tile([128, NJB + 3, HQ, 128], BF, tag="pexp")
                slot = list(range(njb))
                for jb in range(njb):
                    st = psB.tile([128, HQ, 128], F32, tag="st")
                    nc.tensor.matmul(
                        st.rearrange("p h i -> p (h i)"),
                        kT_all[:, jb0 + jb, :],
                        qkT_all[:, tt, :, :].rearrange(
                            "p h i -> p (h i)"),
                        start=True, stop=True)
                    nc.scalar.activation(
                        pexp[:, jb, :, :].rearrange("p h i -> p (h i)"),
                        st.rearrange("p h i -> p (h i)"), AF.Exp)
                    if jb == nhist:
                        nc.vector.tensor_mul(
                            pexp[:, NJB + 2, :, :], pexp[:, jb, :, :],
                            _bcast_mid(masks_sb[:, 1, :], HQ))
                        slot[jb] = NJB + 2
                    elif jb == 0 and tt >= NJB:
                        nc.vector.tensor_mul(
                            pexp[:, NJB + 1, :, :], pexp[:, jb, :, :],
                            _bcast_mid(masks_sb[:, 0, :], HQ))
                        slot[jb] = NJB + 1

                for h in range(HQ):
                    # PV with ones column: col 128 = row sums.  The
                    # y tile also hosts the yT transpose result in cols
                    # 129:257 so both fit one PSUM bank.
                    y_ps = psY.tile([128, 257], F32, tag="y")
                    for jb in range(njb):
                        nc.tensor.matmul(y_ps[:, 0:129],
                                         pexp[:, slot[jb], h, :],
                                         v_all[:, jb0 + jb, :],
                                         start=(jb == 0),
                                         stop=(jb == njb - 1))
                    rsum = attn.tile([128, 1], F32, tag="rsum")
                    nc.vector.reciprocal(rsum, y_ps[:, 128:129])
                    y_sb = attn.tile([128, 128], BF, tag="ysb")
                    if h % 2 == 0:
                        nc.vector.tensor_scalar_mul(y_sb,
                                                    y_ps[:, 0:128],
                                                    rsum)
                    else:
                        nc.scalar.activation(y_sb, y_ps[:, 0:128],
                                             AF.Copy, scale=rsum)
                    # transpose y -> yT
                    nc.tensor.matmul(y_ps[:, 129:257], y_sb, ident_sb,
                                     start=True, stop=True)
                    nc.vector.tensor_copy(yT_all[:, tt, h, :],
                                          y_ps[:, 129:257])

                # out-projection runs one tile LAGGED so the PE
                # never stalls on this tile's y normalization chain
                if tt > 0:
                    _emit_outproj(tt - 1)
            _emit_outproj(NT - 1)


def build_nc(stage=99):
    nc = bacc.Bacc("TRN2", target_bir_lowering=False, debug=False,
                   num_devices=8)
    io = {
        "xT": nc.dram_tensor("xT", [E, T], BF, kind="ExternalInput").ap(),
        "wq": nc.dram_tensor("wq", [E, HQ * D], BF, kind="ExternalInput").ap(),
        "wkvg": nc.dram_tensor("wkvg", [E, 256], BF, kind="ExternalInput").ap(),
        "wo": nc.dram_tensor("wo", [HQ * D, E], BF, kind="ExternalInput").ap(),
        "ve2": nc.dram_tensor("ve2", [T, D], BF, kind="ExternalInput").ap(),
        "cos": nc.dram_tensor("cos", [T, 128], BF, kind="ExternalInput").ap(),
        "sin": nc.dram_tensor("sin", [T, 128], BF, kind="ExternalInput").ap(),
        "masks": nc.dram_tensor("masks", [2, 128, 128], BF,
                                kind="ExternalInput").ap(),
        "ident": nc.dram_tensor("ident", [128, 128], BF,
                                kind="ExternalInput").ap(),
        "out": nc.dram_tensor("out", [T, E], BF, kind="ExternalOutput").ap(),
    }
    with tile.TileContext(nc) as tc:
        _body(tc, io)
    nc.compile()
    return nc


_NC = None


def _get_nc():
    global _NC
    if _NC is None:
        _NC = build_nc()
    return _NC


def _prep_in_maps(x, ve, cos, sin, wq, wk, wv, wo, wgate):
    x = np.asarray(x, dtype=np.float32)
    ve = np.asarray(ve, dtype=np.float32)
    cos1 = np.asarray(cos, np.float32).reshape(T, 64)
    sin1 = np.asarray(sin, np.float32).reshape(T, 64)
    cos2 = np.ascontiguousarray(
        np.concatenate([cos1, cos1], axis=1)).astype(ml_dtypes.bfloat16)
    sin2 = np.ascontiguousarray(
        np.concatenate([sin1, -sin1], axis=1)).astype(ml_dtypes.bfloat16)
    ii = np.arange(128)
    masks = np.ones((2, 128, 128), np.float32)
    # multiplicative post-exp masks, transposed for the P^T[j, i] layout:
    # window-edge block: kill keys j <= i ; causal diag block: kill j > i
    masks[0][ii[:, None] <= ii[None, :]] = 0.0
    masks[1][ii[:, None] > ii[None, :]] = 0.0
    masks = np.ascontiguousarray(masks).astype(ml_dtypes.bfloat16)
    ident = np.eye(128, dtype=ml_dtypes.bfloat16)

    xT_b = [np.ascontiguousarray(x[b].T).astype(ml_dtypes.bfloat16)
            for b in range(B)]
    in_maps = []
    for c in range(8):
        b, g = divmod(c, NKV)
        wq_c = np.ascontiguousarray(
            wq[g * 512:(g + 1) * 512, :].T).astype(ml_dtypes.bfloat16)
        wk_c = wk[g * 128:(g + 1) * 128, :].T
        wv_c = wv[g * 128:(g + 1) * 128, :].T
        wkvg_c = np.ascontiguousarray(
            np.concatenate([wk_c, wv_c], axis=1)).astype(ml_dtypes.bfloat16)
        wo_c = np.ascontiguousarray(
            wo[:, g * 512:(g + 1) * 512].T).astype(ml_dtypes.bfloat16)
        # value-embedding gate folded on the host (tiny 32-ch matmul)
        logit = x[b, :, :32] @ wgate[g]
        gate = 2.0 / (1.0 + np.exp(-logit))
        ve2_c = np.ascontiguousarray(
            gate[:, None] * ve[b, :, g * 128:(g + 1) * 128]).astype(
                ml_dtypes.bfloat16)
        in_maps.append({
            "xT": xT_b[b], "wq": wq_c, "wkvg": wkvg_c, "wo": wo_c,
            "ve2": ve2_c, "cos": cos2, "sin": sin2, "masks": masks,
            "ident": ident,
        })
    return in_maps


def kernel(x, ve, cos, sin, wq, wk, wv, wo, wgate, window_size=512,
           _trace=False):
    assert int(window_size) == W, f"kernel hardcodes window {W}"
    wq = np.asarray(wq, np.float32)
    wk = np.asarray(wk, np.float32)
    wv = np.asarray(wv, np.float32)
    wo = np.asarray(wo, np.float32)
    wgate = np.asarray(wgate, np.float32)
    in_maps = _prep_in_maps(x, ve, cos, sin, wq, wk, wv, wo, wgate)
    nc = _get_nc()
    res = bass_utils.run_bass_kernel_spmd(
        nc, in_maps, core_ids=list(range(8)), trace=_trace)
    out = np.empty((B, T, E), np.float32)
    for b in range(B):
        acc = res.results[b * NKV]["out"].astype(np.float32).copy()
        for g in range(1, NKV):
            acc += res.results[b * NKV + g]["out"]
        out[b] = acc
    if _trace:
        kernel.last_results = res
    return out


# revision 27
# speedup vs baseline: 1.2177x; 1.0015x over previous
"""Trainium2 Bass kernel for NanochatAttention (sliding-window GQA attention).

Sharding: 8 cores = (batch b in {0,1}) x (kv-group g in {0..3}).
Each core handles one batch's full sequence for one KV head and its 4 Q heads:
projections, RoPE + QK RMS-norm, value-embedding gate, 512-window causal
attention, and the row-parallel out-projection slice -> partial [T, E] output.
Host sums the 4 partials per batch at unshard time.

v5 highlights (evidence-driven, see trace history):
  * RMS rsqrt = ACT Sqrt + DVE reciprocal.  Square/Sqrt/Copy share ONE
    activation-table set, Exp (phase B) is the only other set -> 2 table
    loads total.  (v2's Ln+Exp rsqrt alternated two sets: 33 loads,
    ~2.6us/tile of Scalar-engine stall that backed up the PE.)
  * DMA: few big issues (each sync dma_start costs ~0.6us of sync-engine
    time).  Weights + the first 256 t-cols of x go in 4-slab groups so
    tile-0's projection chain starts as soon as group 0 lands; x remainder
    streams in 256-col slices ahead of the per-tile compute.
  * Out-projection lags one tile and stages per-512-col chunks (3 DVE /
    1 ACT copies).  NOTE measured dead ends: DMA straight from PSUM is
    not allowed (dma_start src must be SBUF); interleaving the staging
    copies between y-chain heads + grouped 4-slab weight DMAs DID fill
    the PE gaps (43us idle vs 59us) but made every matmul ~20% slower
    from SBUF port contention - net 274us vs 256us.  Less concurrency
    won here.
  * fp8 DoubleRow was tried (v4) and reverted: with --enable-ldw-opt=false
    every matmul pays its own LDWEIGHTS, so 3x more 256-col instructions
    lose to bf16's 512-col streams (314us vs 256us).
  * tensor_tensor_reduce / Pool-engine ops crash this device - avoided.

Attention computes TRANSPOSED scores ST[j, i] = k_j . q_i directly
(stationary = kT block, moving = all 4 heads' scaled qT), so the exp
output IS P^T in SBUF (no PE transpose matmuls for P).  Row sums come
from a ones-column appended to V (PV matmul streams 129 cols; col 128
accumulates sum_j P[i, j]).  Softmax normalization is applied
per-partition while copying the PV accumulator out of PSUM; an identity-
transpose per (tile, head) yields yT for the out-projection.
"""

import numpy as np
import ml_dtypes

import concourse.bass as bass
import concourse.bacc as bacc
import concourse.tile as tile
from concourse import mybir
from concourse import bass_utils

BF = mybir.dt.bfloat16
F32 = mybir.dt.float32
AF = mybir.ActivationFunctionType
ALU = mybir.AluOpType

B = 2
T = 2048
E = 2048
D = 128          # head dim
HQ = 4           # q heads per core (one kv group)
NKV = 4
NT = T // 128    # 16 t-tiles
NE = E // 128    # 16 e-tiles
W = 512          # sliding window
NJB = W // 128   # history blocks
EPS = float(np.finfo(np.float32).eps)


def _bcast_mid(ap, n):
    """Insert a step-0 dim after the partition dim: [p, w] -> [p, n, w]."""
    return bass.AP(tensor=ap.tensor, offset=ap.offset,
                   ap=[ap.ap[0], [0, n], *ap.ap[1:]])


def _half_swap(ap2d, nmid):
    """[p, nmid*128] -> [p, nmid, 2, 64] view with the 64-halves swapped."""
    return bass.AP(tensor=ap2d.tensor, offset=ap2d.offset + 64,
                   ap=[ap2d.ap[0], [128, nmid], [-64, 2], [1, 64]])


def _body(tc, io):
    nc = tc.nc
    xT, wq, wkvg, wo, ve2, cosd, sind, masks, ident, out = (
        io["xT"], io["wq"], io["wkvg"], io["wo"], io["ve2"], io["cos"],
        io["sin"], io["masks"], io["ident"], io["out"])

    with (
        tc.tile_pool(name="const", bufs=1) as cpool,
        tc.tile_pool(name="state", bufs=1) as state,
    ):
        ident_sb = cpool.tile([128, 128], BF, tag="ident")
        nc.sync.dma_start(ident_sb, ident)
        cos_sb = cpool.tile([128, NT, 128], BF, tag="cos")
        sin_sb = cpool.tile([128, NT, 128], BF, tag="sin")
        ve_sb = cpool.tile([128, NT, D], BF, tag="ve")
        cosr = cosd.rearrange("(t p) h -> p t h", p=128)
        sinr = sind.rearrange("(t p) h -> p t h", p=128)
        ver = ve2.rearrange("(t p) d -> p t d", p=128)
        nc.sync.dma_start(cos_sb[:, 0:2, :], cosr[:, 0:2, :])
        nc.sync.dma_start(sin_sb[:, 0:2, :], sinr[:, 0:2, :])
        nc.sync.dma_start(ve_sb[:, 0:2, :], ver[:, 0:2, :])

        wqkvg_sb = cpool.tile([128, NE, 768], BF, tag="wqkvg")
        wqd = wq.rearrange("(e p) f -> p e f", p=128)
        wkvgd = wkvg.rearrange("(e p) f -> p e f", p=128)
        xTr = xT.rearrange("(e p) t -> p e t", p=128)
        with tc.tile_pool(name="xp", bufs=1) as xp:
            xT_sb = xp.tile([128, NE, T], BF, tag="xT")
            # weights + the first 256 t-cols of x in 4-slab groups: the
            # measured ~2us/slab arrival cadence was ISSUE-limited (3
            # dma_starts x 0.6us sync time per slab), so batching 4 slabs
            # per issue feeds the paired projection stream without gaps.
            # slab 0 goes alone so the first matmul fires ~3us earlier
            # than waiting on a full 1MB 4-slab group.
            for es in (slice(0, 1), slice(1, 4), slice(4, 8),
                       slice(8, 12), slice(12, 16)):
                nc.sync.dma_start(wqkvg_sb[:, es, 0:512], wqd[:, es])
                nc.sync.dma_start(wqkvg_sb[:, es, 512:768], wkvgd[:, es])
                nc.sync.dma_start(xT_sb[:, es, 0:256], xTr[:, es, 0:256])
            # x slices for tiles 2-5 BEFORE the small-tensor remainder:
            # the measured 6.8us PE stall at t~44us was phase A waiting
            # for slice tb=1 behind the cos/sin/ve bytes.
            for tb in (1, 2):
                cs = slice(tb * 256, (tb + 1) * 256)
                nc.sync.dma_start(xT_sb[:, :, cs], xTr[:, :, cs])
            nc.sync.dma_start(cos_sb[:, 2:, :], cosr[:, 2:, :])
            nc.sync.dma_start(sin_sb[:, 2:, :], sinr[:, 2:, :])
            nc.sync.dma_start(ve_sb[:, 2:, :], ver[:, 2:, :])
            for tb in range(3, T // 256):
                cs = slice(tb * 256, (tb + 1) * 256)
                nc.sync.dma_start(xT_sb[:, :, cs], xTr[:, :, cs])
            masks_sb = cpool.tile([128, 2, 128], BF, tag="masks")
            nc.sync.dma_start(masks_sb, masks.rearrange("m p j -> p m j"))
            wo_sb = cpool.tile([128, HQ, E], BF, tag="wo")
            nc.sync.dma_start(wo_sb, wo.rearrange("(h p) e -> p h e", p=128))

            # ---- whole-sequence state ------------------------------------
            qkT_all = state.tile([128, NT, HQ, 128], BF, tag="qkT")
            kT_all = state.tile([128, NT, 128], BF, tag="kT")
            v_all = state.tile([128, NT, 129], BF, tag="v")
            yT_all = state.tile([128, NT, HQ, 128], BF, tag="yT")

            # ones column for PV row sums (independent of v_all's data cols)
            nc.vector.tensor_scalar(
                v_all[:, :, 128:129].rearrange("p a b -> p (a b)"),
                ident_sb[:, 0:NT], 0.0, 1.0, op0=ALU.mult, op1=ALU.add)

            # ============ PHASE A: proj + rope + rms + transposes =========
            with (
                tc.tile_pool(name="workA", bufs=3) as work,
                tc.tile_pool(name="psA_q", bufs=4, space="PSUM") as psA_q,
                tc.tile_pool(name="psA_kvg", bufs=2, space="PSUM") as psA_kvg,
                tc.tile_pool(name="psA_tr", bufs=1, space="PSUM") as psA_tr,
            ):
                def _emit_transposes(ptt, qros_p, krms_p):
                    trp = psA_tr.tile([128, HQ + 1, 128], F32, tag="trp")
                    for h in range(HQ):
                        nc.tensor.matmul(trp[:, h, :], qros_p[:, h, :],
                                         ident_sb, start=True, stop=True)
                    nc.tensor.matmul(trp[:, HQ, :], krms_p, ident_sb,
                                     start=True, stop=True)
                    # qT copy on ACT (Copy is a filler in every table set)
                    nc.scalar.activation(
                        qkT_all[:, ptt, :, :].rearrange("p a b -> p (a b)"),
                        trp[:, 0:HQ, :].rearrange("p a b -> p (a b)"),
                        AF.Copy)
                    nc.vector.tensor_copy(kT_all[:, ptt, :], trp[:, HQ, :])

                lagq = []

                def _proj_post(tt, psq, pskvg):
                    ts = slice(tt * 128, (tt + 1) * 128)
                    # v = v_proj + gated ve (gate pre-applied on host)
                    nc.vector.tensor_add(v_all[:, tt, 0:128],
                                         pskvg[:, 128:256], ve_sb[:, tt, :])

                    # cast projections to bf16, then RoPE in fast DVE modes
                    qb = work.tile([128, HQ * D], BF, tag="qb")
                    nc.vector.tensor_copy(qb, psq)
                    kb = work.tile([128, D], BF, tag="kb")
                    nc.vector.tensor_copy(kb, pskvg[:, 0:128])

                    cosq = _bcast_mid(cos_sb[:, tt, :], HQ)
                    sinq = _bcast_mid(sin_sb[:, tt, :], HQ)
                    qro = work.tile([128, HQ, D], BF, tag="qro")
                    rb = work.tile([128, HQ, D], BF, tag="rb")
                    nc.vector.tensor_mul(
                        qro, qb.rearrange("p (h d) -> p h d", h=HQ), cosq)
                    nc.vector.tensor_mul(
                        rb.rearrange("p h (s x) -> p h s x", s=2),
                        _half_swap(qb[:], HQ),
                        sinq.rearrange("p h (s x) -> p h s x", s=2))
                    nc.vector.tensor_add(qro, qro, rb)

                    kro = work.tile([128, D], BF, tag="kro")
                    kb2 = work.tile([128, D], BF, tag="kb2")
                    nc.vector.tensor_mul(kro, kb, cos_sb[:, tt, :])
                    nc.vector.tensor_mul(
                        kb2.rearrange("p (s x) -> p s x", s=2),
                        _half_swap(kb[:], 1),
                        sin_sb[:, tt, :].rearrange("p (s x) -> p s x", s=2))
                    nc.vector.tensor_add(kro, kro, kb2)

                    # RMS sums of squares (ACT Square + accum)
                    sq = work.tile([128, (HQ + 1) * D], BF, tag="sq")
                    qkss = work.tile([128, HQ + 1], F32, tag="qkss")
                    for h in range(HQ):
                        nc.scalar.activation(sq[:, h * D:(h + 1) * D],
                                             qro[:, h, :], AF.Square,
                                             accum_out=qkss[:, h:h + 1])
                    nc.scalar.activation(sq[:, HQ * D:], kro, AF.Square,
                                         accum_out=qkss[:, HQ:HQ + 1])
                    # rsqrt = DVE reciprocal(ACT Sqrt); the k column folds
                    # the sqrt(D) score scale via a 1/D pre-multiply.
                    tmp5 = work.tile([128, HQ + 1], F32, tag="tmp5")
                    nc.vector.tensor_scalar_add(tmp5[:, 0:HQ],
                                                qkss[:, 0:HQ],
                                                float(128.0 * EPS))
                    nc.vector.tensor_scalar(tmp5[:, HQ:HQ + 1],
                                            qkss[:, HQ:HQ + 1],
                                            float(1.0 / 128.0), EPS,
                                            op0=ALU.mult, op1=ALU.add)
                    rt5 = work.tile([128, HQ + 1], F32, tag="rt5")
                    nc.scalar.activation(rt5, tmp5, AF.Sqrt)
                    sc5 = work.tile([128, HQ + 1], F32, tag="sc5")
                    nc.vector.reciprocal(sc5, rt5)

                    # krms = kro * sc5_k ; qros_h = qro_h * sc5_h (DVE)
                    krms = work.tile([128, D], BF, tag="krms")
                    nc.vector.tensor_scalar_mul(krms, kro,
                                                sc5[:, HQ:HQ + 1])
                    qros = work.tile([128, HQ, D], BF, tag="qros")
                    for h in range(HQ):
                        nc.vector.tensor_scalar_mul(qros[:, h, :],
                                                    qro[:, h, :],
                                                    sc5[:, h:h + 1])

                    # transposes run TWO tiles lagged so the PE never
                    # waits on the DVE/ACT chain
                    lagq.append((tt, qros, krms))
                    if len(lagq) > 2:
                        _emit_transposes(*lagq.pop(0))

                # projections run in tile PAIRS with the q matmuls
                # interleaved per contraction slab: during the initial
                # weight stream each arriving slab unlocks ~1.3us of PE
                # work instead of ~0.65us (tile-0-only), halving the
                # measured per-slab chase gaps at t~15-44us.
                for tp in range(0, NT, 2):
                    ts0 = slice(tp * 128, (tp + 1) * 128)
                    ts1 = slice((tp + 1) * 128, (tp + 2) * 128)
                    psq0 = psA_q.tile([128, 512], F32, tag="psq")
                    psq1 = psA_q.tile([128, 512], F32, tag="psq")
                    kvg0 = psA_kvg.tile([128, 256], F32, tag="pskvg")
                    kvg1 = psA_kvg.tile([128, 256], F32, tag="pskvg")
                    for e in range(NE):
                        nc.tensor.matmul(psq0, xT_sb[:, e, ts0],
                                         wqkvg_sb[:, e, 0:512],
                                         start=(e == 0), stop=(e == NE - 1))
                        nc.tensor.matmul(psq1, xT_sb[:, e, ts1],
                                         wqkvg_sb[:, e, 0:512],
                                         start=(e == 0), stop=(e == NE - 1))
                    for e in range(NE):
                        nc.tensor.matmul(kvg0, xT_sb[:, e, ts0],
                                         wqkvg_sb[:, e, 512:768],
                                         start=(e == 0), stop=(e == NE - 1))
                        nc.tensor.matmul(kvg1, xT_sb[:, e, ts1],
                                         wqkvg_sb[:, e, 512:768],
                                         start=(e == 0), stop=(e == NE - 1))
                    _proj_post(tp, psq0, kvg0)
                    _proj_post(tp + 1, psq1, kvg1)
                for item in lagq:
                    _emit_transposes(*item)

        # ============= PHASE B: attention ============================
        with (
            tc.tile_pool(name="attn", bufs=3) as attn,
            # outc MUST stay at 2: bufs=3 measured 284us and bufs=4 281us
            # (vs 252us) - deeper staging lets the out-proj copies run
            # ahead and the added engine concurrency slows every matmul
            # stream via SBUF port contention.
            tc.tile_pool(name="outc", bufs=2) as outc,
            tc.tile_pool(name="psB", bufs=3, space="PSUM") as psB,
            tc.tile_pool(name="psY", bufs=2, space="PSUM") as psY,
            tc.tile_pool(name="psC", bufs=3, space="PSUM") as psC,
        ):
            def _emit_outproj(ptt):
                # four 512-col chunks, each DMA'd as soon as it lands
                pts = slice(ptt * 128, (ptt + 1) * 128)
                for ec in range(4):
                    ops = psC.tile([128, 512], F32, tag="ops")
                    for h in range(HQ):
                        nc.tensor.matmul(
                            ops, yT_all[:, ptt, h, :],
                            wo_sb[:, h, ec * 512:(ec + 1) * 512],
                            start=(h == 0), stop=(h == HQ - 1))
                    osb = outc.tile([128, 512], BF, tag="osb")
                    if ec == 3:
                        nc.scalar.activation(osb, ops, AF.Copy)
                    else:
                        nc.vector.tensor_copy(osb, ops)
                    nc.sync.dma_start(out[pts, ec * 512:(ec + 1) * 512],
                                      osb)

            for tt in range(NT):
                njb = min(tt, NJB) + 1
                jb0 = tt - (njb - 1)
                nhist = njb - 1
                # scores for ALL 4 heads per key block: one 512-col
                # matmul (stationary kT block, moving all heads' qT),
                # exp'd per block.  Masks are applied POST-exp as
                # multiplicative 0/1 bf16 on the DVE.
                pexp = attn.tile([128, NJB + 3, HQ, 128], BF, tag="pexp")
                slot = list(range(njb))
                for jb in range(njb):
                    st = psB.tile([128, HQ, 128], F32, tag="st")
                    nc.tensor.matmul(
                        st.rearrange("p h i -> p (h i)"),
                        kT_all[:, jb0 + jb, :],
                        qkT_all[:, tt, :, :].rearrange(
                            "p h i -> p (h i)"),
                        start=True, stop=True)
                    nc.scalar.activation(
                        pexp[:, jb, :, :].rearrange("p h i -> p (h i)"),
                        st.rearrange("p h i -> p (h i)"), AF.Exp)
                    if jb == nhist:
                        nc.vector.tensor_mul(
                            pexp[:, NJB + 2, :, :], pexp[:, jb, :, :],
                            _bcast_mid(masks_sb[:, 1, :], HQ))
                        slot[jb] = NJB + 2
                    elif jb == 0 and tt >= NJB:
                        nc.vector.tensor_mul(
                            pexp[:, NJB + 1, :, :], pexp[:, jb, :, :],
                            _bcast_mid(masks_sb[:, 0, :], HQ))
                        slot[jb] = NJB + 1

                for h in range(HQ):
                    # PV with ones column: col 128 = row sums.  The
                    # y tile also hosts the yT transpose result in cols
                    # 129:257 so both fit one PSUM bank.
                    y_ps = psY.tile([128, 257], F32, tag="y")
                    for jb in range(njb):
                        nc.tensor.matmul(y_ps[:, 0:129],
                                         pexp[:, slot[jb], h, :],
                                         v_all[:, jb0 + jb, :],
                                         start=(jb == 0),
                                         stop=(jb == njb - 1))
                    rsum = attn.tile([128, 1], F32, tag="rsum")
                    nc.vector.reciprocal(rsum, y_ps[:, 128:129])
                    y_sb = attn.tile([128, 128], BF, tag="ysb")
                    if h % 2 == 0:
                        nc.vector.tensor_scalar_mul(y_sb,
                                                    y_ps[:, 0:128],
                                                    rsum)
                    else:
                        nc.scalar.activation(y_sb, y_ps[:, 0:128],
                                             AF.Copy, scale=rsum)
                    # transpose y -> yT
                    nc.tensor.matmul(y_ps[:, 129:257], y_sb, ident_sb,
                                     start=True, stop=True)
                    nc.vector.tensor_copy(yT_all[:, tt, h, :],
                                          y_ps[:, 129:257])

                # out-projection runs one tile LAGGED so the PE
                # never stalls on this tile's y normalization chain
                if tt > 0:
                    _emit_outproj(tt - 1)
            _emit_outproj(NT - 1)


def build_nc(stage=99):
    nc = bacc.Bacc("TRN2", target_bir_lowering=False, debug=False,
                   num_devices=8)
    io = {
        "xT": nc.dram_tensor("xT", [E, T], BF, kind="ExternalInput").ap(),
        "wq": nc.dram_tensor("wq", [E, HQ * D], BF, kind="ExternalInput").ap(),
        "wkvg": nc.dram_tensor("wkvg", [E, 256], BF, kind="ExternalInput").ap(),
        "wo": nc.dram_tensor("wo", [HQ * D, E], BF, kind="ExternalInput").ap(),
        "ve2": nc.dram_tensor("ve2", [T, D], BF, kind="ExternalInput").ap(),
        "cos": nc.dram_tensor("cos", [T, 128], BF, kind="ExternalInput").ap(),
        "sin": nc.dram_tensor("sin", [T, 128], BF, kind="ExternalInput").ap(),
        "masks": nc.dram_tensor("masks", [2, 128, 128], BF,
                                kind="ExternalInput").ap(),
        "ident": nc.dram_tensor("ident", [128, 128], BF,
                                kind="ExternalInput").ap(),
        "out": nc.dram_tensor("out", [T, E], BF, kind="ExternalOutput").ap(),
    }
    with tile.TileContext(nc) as tc:
        _body(tc, io)
    nc.compile()
    return nc


_NC = None


def _get_nc():
    global _NC
    if _NC is None:
        _NC = build_nc()
    return _NC


def _prep_in_maps(x, ve, cos, sin, wq, wk, wv, wo, wgate):
    x = np.asarray(x, dtype=np.float32)
    ve = np.asarray(ve, dtype=np.float32)
    cos1 = np.asarray(cos, np.float32).reshape(T, 64)
    sin1 = np.asarray(sin, np.float32).reshape(T, 64)
    cos2 = np.ascontiguousarray(
        np.concatenate([cos1, cos1], axis=1)).astype(ml_dtypes.bfloat16)
    sin2 = np.ascontiguousarray(
        np.concatenate([sin1, -sin1], axis=1)).astype(ml_dtypes.bfloat16)
    ii = np.arange(128)
    masks = np.ones((2, 128, 128), np.float32)
    # multiplicative post-exp masks, transposed for the P^T[j, i] layout:
    # window-edge block: kill keys j <= i ; causal diag block: kill j > i
    masks[0][ii[:, None] <= ii[None, :]] = 0.0
    masks[1][ii[:, None] > ii[None, :]] = 0.0
    masks = np.ascontiguousarray(masks).astype(ml_dtypes.bfloat16)
    ident = np.eye(128, dtype=ml_dtypes.bfloat16)

    xT_b = [np.ascontiguousarray(x[b].T).astype(ml_dtypes.bfloat16)
            for b in range(B)]
    in_maps = []
    for c in range(8):
        b, g = divmod(c, NKV)
        wq_c = np.ascontiguousarray(
            wq[g * 512:(g + 1) * 512, :].T).astype(ml_dtypes.bfloat16)
        wk_c = wk[g * 128:(g + 1) * 128, :].T
        wv_c = wv[g * 128:(g + 1) * 128, :].T
        wkvg_c = np.ascontiguousarray(
            np.concatenate([wk_c, wv_c], axis=1)).astype(ml_dtypes.bfloat16)
        wo_c = np.ascontiguousarray(
            wo[:, g * 512:(g + 1) * 512].T).astype(ml_dtypes.bfloat16)
        # value-embedding gate folded on the host (tiny 32-ch matmul)
        logit = x[b, :, :32] @ wgate[g]
        gate = 2.0 / (1.0 + np.exp(-logit))
        ve2_c = np.ascontiguousarray(
            gate[:, None] * ve[b, :, g * 128:(g + 1) * 128]).astype(
                ml_dtypes.bfloat16)
        in_maps.append({
            "xT": xT_b[b], "wq": wq_c, "wkvg": wkvg_c, "wo": wo_c,
            "ve2": ve2_c, "cos": cos2, "sin": sin2, "masks": masks,
            "ident": ident,
        })
    return in_maps


def kernel(x, ve, cos, sin, wq, wk, wv, wo, wgate, window_size=512,
           _trace=False):
    assert int(window_size) == W, f"kernel hardcodes window {W}"
    wq = np.asarray(wq, np.float32)
    wk = np.asarray(wk, np.float32)
    wv = np.asarray(wv, np.float32)
    wo = np.asarray(wo, np.float32)
    wgate = np.asarray(wgate, np.float32)
    in_maps = _prep_in_maps(x, ve, cos, sin, wq, wk, wv, wo, wgate)
    nc = _get_nc()
    res = bass_utils.run_bass_kernel_spmd(
        nc, in_maps, core_ids=list(range(8)), trace=_trace)
    out = np.empty((B, T, E), np.float32)
    for b in range(B):
        acc = res.results[b * NKV]["out"].astype(np.float32).copy()
        for g in range(1, NKV):
            acc += res.results[b * NKV + g]["out"]
        out[b] = acc
    if _trace:
        kernel.last_results = res
    return out
